# revision 3
# baseline (speedup 1.0000x reference)
"""Trainium2 Bass kernel for nn_Attention_45414984188085.

Reference (per batch b):
    h   = tanh(q[b] @ W1 + k[b] @ W2)          # [S, H]
    sc  = h @ V                                # [S, H]
    w   = softmax(sc, axis=-1)                 # softmax over feature dim
    val = sum_s w[s, :] * q[b, s, :]           # [D]
    returns (values [B, D], weights [B, S, D])

Sharding: data-parallel over batch B=8 across the 8 NeuronCores; each core
computes its batch fully (no collectives).

Per-core dataflow (big matmuls in float32r - fp32-class precision at
1 cycle/row PE throughput):
  mm1:  hT[h, s]  = sum_d W1[d, h] * qT[d, s]  (lhsT = W1 tile, rhs = qT tile)
        tanh fused on ScalarE with per-partition bias = (k @ W2)[h]
  mm2:  sc[s, v]  = sum_h hT[h, s] * V[h, v]   (lhsT = hT tile,  rhs = V tile)
        -> natural [S, v] layout, softmax over the free dim
  softmax: exp on ScalarE with accum_out giving per-row sums; reciprocal +
        scale on VectorE; weights DMA'd out in natural layout.
  values: p = w * q (VectorE), then partition-reduce via PE matmul with a
        ones[128,1] stationary vector (bf16: f32r disallows M=1/N=1),
        accumulated in PSUM across all S.

DMA: startup weight loads are striped across the three DGE descriptor
streams (sync HWDGE: q-transposed tiles; scalar HWDGE: W1; gpsimd SWDGE:
kT/W2/V) so the first matmul isn't queued behind 10 MB of weights.
"""

import numpy as np
import ml_dtypes
from contextlib import ExitStack

try:
    import concourse.bass as bass  # noqa: F401
except ImportError:  # pragma: no cover - defensive for fresh grading dirs
    import sys

    sys.path.insert(0, "/root/.axon_site/_ro/trn_rl_repo")

import concourse.bass as bass
import concourse.tile as tile
from concourse import bacc, mybir
from concourse.bass_utils import run_bass_kernel_spmd

P = 128
B, S, D = 8, 4096, 1024
ND = D // P           # 8 blocks of 128 along D/H/v
SGRP = 512            # seq columns processed per mm1 group
F32 = mybir.dt.float32
F32R = mybir.dt.float32r
BF16 = mybir.dt.bfloat16
TANH = mybir.ActivationFunctionType.Tanh
EXP = mybir.ActivationFunctionType.Exp


def build_program(s_len=S):
    """Build the per-core Bass program (same program on all 8 cores)."""
    nsg = s_len // SGRP
    nsb = s_len // P

    nc = bacc.Bacc("TRN2", target_bir_lowering=False, debug=False)

    qT = nc.dram_tensor("qT", [D, s_len], F32R, kind="ExternalInput")
    qn = nc.dram_tensor("qn", [s_len, D], BF16, kind="ExternalInput")
    w1 = nc.dram_tensor("w1", [D, D], F32R, kind="ExternalInput")
    vv = nc.dram_tensor("vv", [D, D], F32R, kind="ExternalInput")
    w2 = nc.dram_tensor("w2", [D, D], BF16, kind="ExternalInput")
    kT = nc.dram_tensor("kT", [P, ND], BF16, kind="ExternalInput")
    wout = nc.dram_tensor("wout", [s_len, D], F32, kind="ExternalOutput")
    vout = nc.dram_tensor("vout", [1, D], F32, kind="ExternalOutput")

    with tile.TileContext(nc) as tc, ExitStack() as ctx:
        const = ctx.enter_context(tc.tile_pool(name="const", bufs=1))
        w1p = ctx.enter_context(tc.tile_pool(name="w1p", bufs=1))
        vp = ctx.enter_context(tc.tile_pool(name="vp", bufs=1))
        w2p = ctx.enter_context(tc.tile_pool(name="w2p", bufs=1))
        qtsp = ctx.enter_context(tc.tile_pool(name="qtsp", bufs=16))
        htsp = ctx.enter_context(tc.tile_pool(name="htsp", bufs=16))
        expp = ctx.enter_context(tc.tile_pool(name="expp", bufs=3))
        wsbp = ctx.enter_context(tc.tile_pool(name="wsbp", bufs=3))
        qnp = ctx.enter_context(tc.tile_pool(name="qnp", bufs=4))
        pp = ctx.enter_context(tc.tile_pool(name="pp", bufs=3))
        smp = ctx.enter_context(tc.tile_pool(name="smp", bufs=4))
        ps1 = ctx.enter_context(tc.tile_pool(name="ps1", bufs=2, space="PSUM"))
        ps2 = ctx.enter_context(tc.tile_pool(name="ps2", bufs=2, space="PSUM"))
        psv = ctx.enter_context(tc.tile_pool(name="psv", bufs=1, space="PSUM"))

        ones_t = const.tile([P, 1], BF16, name="ones_t")
        nc.vector.memset(ones_t[:], 1.0)

        # gpsimd (SWDGE) stream: kT, W2 rows (feed the h_k chain), then V rows
        kt_sb = const.tile([P, ND], BF16, name="kt_sb")
        nc.gpsimd.dma_start(kt_sb[:], kT.ap())
        w2_t = []
        for i in range(ND):
            w2t = w2p.tile([P, D], BF16, name=f"w2t_{i}", tag=f"w2_{i}")
            nc.gpsimd.dma_start(w2t[:], w2.ap()[i * P:(i + 1) * P, :])
            w2_t.append(w2t)

        # sync (HWDGE) stream: first group of qT tiles before anything else
        qts_all = {}
        for d in range(ND):
            qt = qtsp.tile([P, SGRP], F32R, name=f"qts_0_{d}", tag="qts")
            nc.sync.dma_start(qt[:], qT.ap()[d * P:(d + 1) * P, 0:SGRP])
            qts_all[(0, d)] = qt

        # scalar (HWDGE) stream: W1 rows
        w1_t = []
        for i in range(ND):
            w1t = w1p.tile([P, D], F32R, name=f"w1t_{i}", tag=f"w1_{i}")
            nc.scalar.dma_start(w1t[:], w1.ap()[i * P:(i + 1) * P, :])
            w1_t.append(w1t)

        # h_k = k @ W2 as a [128, ND] column stack (h on partitions)
        hk_ps = ps1.tile([P, ND], F32, name="hk_ps", tag="m1")
        for h in range(ND):
            for d in range(ND):
                nc.tensor.matmul(
                    hk_ps[:, h:h + 1],
                    w2_t[d][:, h * P:(h + 1) * P],
                    kt_sb[:, d:d + 1],
                    start=(d == 0), stop=(d == ND - 1),
                )
        hk_sb = const.tile([P, ND], F32, name="hk_sb")
        nc.vector.tensor_copy(hk_sb[:], hk_ps[:])

        # V rows ride the gpsimd stream after W2 (needed from first mm2 on)
        v_t = []
        for i in range(ND):
            vt = vp.tile([P, D], F32R, name=f"vt_{i}", tag=f"v_{i}")
            nc.gpsimd.dma_start(vt[:], vv.ap()[i * P:(i + 1) * P, :])
            v_t.append(vt)

        vps0 = psv.tile([1, 512], F32, name="vps0", tag="vps0")
        vps1 = psv.tile([1, 512], F32, name="vps1", tag="vps1")
        vps = [vps0, vps1]

        for g in range(nsg):
            qts = []
            for d in range(ND):
                if (g, d) in qts_all:
                    qts.append(qts_all[(g, d)])
                    continue
                qt = qtsp.tile([P, SGRP], F32R, name=f"qts_{g}_{d}", tag="qts")
                nc.sync.dma_start(
                    qt[:], qT.ap()[d * P:(d + 1) * P, g * SGRP:(g + 1) * SGRP]
                )
                qts.append(qt)

            hts = []
            for h in range(ND):
                m1 = ps1.tile([P, SGRP], F32, name=f"m1_{g}_{h}", tag="m1")
                for d in range(ND):
                    nc.tensor.matmul(
                        m1[:], w1_t[d][:, h * P:(h + 1) * P], qts[d][:],
                        start=(d == 0), stop=(d == ND - 1),
                    )
                ht = htsp.tile([P, SGRP], F32R, name=f"ht_{g}_{h}", tag="ht")
                nc.scalar.activation(ht[:], m1[:], TANH, bias=hk_sb[:, h:h + 1])
                hts.append(ht)

            for sb in range(SGRP // P):
                s_blk = g * (SGRP // P) + sb
                m2 = ps2.tile([P, D], F32, name=f"m2_{s_blk}", tag="m2")
                for h in range(ND):
                    lhs = hts[h][:, sb * P:(sb + 1) * P]
                    for vh in range(2):
                        nc.tensor.matmul(
                            m2[:, vh * 512:(vh + 1) * 512], lhs,
                            v_t[h][:, vh * 512:(vh + 1) * 512],
                            start=(h == 0), stop=(h == ND - 1),
                        )
                exp_t = expp.tile([P, D], F32, name=f"exp_{s_blk}", tag="exp")
                sum_t = smp.tile([P, 1], F32, name=f"sum_{s_blk}", tag="sum")
                nc.scalar.activation(exp_t[:], m2[:], EXP, accum_out=sum_t[:])
                rec_t = smp.tile([P, 1], F32, name=f"rec_{s_blk}", tag="rec")
                nc.vector.reciprocal(rec_t[:], sum_t[:])
                w_t = wsbp.tile([P, D], F32, name=f"w_{s_blk}", tag="w")
                nc.vector.tensor_scalar_mul(w_t[:], exp_t[:], rec_t[:])
                nc.sync.dma_start(wout.ap()[s_blk * P:(s_blk + 1) * P, :], w_t[:])

                qn_t = qnp.tile([P, D], BF16, name=f"qn_{s_blk}", tag="qn")
                nc.sync.dma_start(qn_t[:], qn.ap()[s_blk * P:(s_blk + 1) * P, :])
                p_t = pp.tile([P, D], BF16, name=f"p_{s_blk}", tag="p")
                nc.vector.tensor_mul(p_t[:], w_t[:], qn_t[:])
                for vh in range(2):
                    nc.tensor.matmul(
                        vps[vh][:], ones_t[:], p_t[:, vh * 512:(vh + 1) * 512],
                        start=(s_blk == 0), stop=(s_blk == nsb - 1),
                        skip_group_check=True,
                    )

        vals_sb = const.tile([1, D], F32, name="vals_sb")
        nc.vector.tensor_copy(vals_sb[:, 0:512], vps0[:])
        nc.vector.tensor_copy(vals_sb[:, 512:1024], vps1[:])
        nc.sync.dma_start(vout.ap(), vals_sb[:])

    nc.compile()
    return nc


def make_in_maps(q, k, W1, W2, V, s_len=S):
    q = np.asarray(q, dtype=np.float32)
    k = np.asarray(k, dtype=np.float32)
    W1 = np.ascontiguousarray(np.asarray(W1, dtype=np.float32))
    W2 = np.ascontiguousarray(np.asarray(W2, dtype=np.float32))
    V = np.ascontiguousarray(np.asarray(V, dtype=np.float32))
    w2_bf = W2.astype(ml_dtypes.bfloat16)
    in_maps = []
    for c in range(B):
        qc = q[c, :s_len, :]
        in_maps.append({
            "qT": np.ascontiguousarray(qc.T),
            "qn": qc.astype(ml_dtypes.bfloat16),
            "w1": W1,
            "vv": V,
            "w2": w2_bf,
            "kT": np.ascontiguousarray(k[c].reshape(ND, P).T).astype(ml_dtypes.bfloat16),
        })
    return in_maps


_cached = {}


def run(q, k, W1, W2, V, trace=False):
    if "nc" not in _cached:
        _cached["nc"] = build_program(S)
    nc = _cached["nc"]
    in_maps = make_in_maps(q, k, W1, W2, V)
    res = run_bass_kernel_spmd(nc, in_maps, core_ids=list(range(B)), trace=trace)
    values = np.stack([res.results[c]["vout"][0] for c in range(B)])
    weights = np.stack([res.results[c]["wout"] for c in range(B)])
    return values, weights, res


def kernel(q, k, W1, W2, V):
    values, weights, _ = run(q, k, W1, W2, V)
    return values, weights


# revision 4
# speedup vs baseline: 1.0094x; 1.0094x over previous
"""Trainium2 Bass kernel for nn_Attention_45414984188085.

Reference (per batch b):
    h   = tanh(q[b] @ W1 + k[b] @ W2)          # [S, H]
    sc  = h @ V                                # [S, H]
    w   = softmax(sc, axis=-1)                 # softmax over feature dim
    val = sum_s w[s, :] * q[b, s, :]           # [D]
    returns (values [B, D], weights [B, S, D])

Sharding: data-parallel over batch B=8 across the 8 NeuronCores; each core
computes its batch fully (no collectives).

Per-core dataflow (big matmuls in float32r - fp32-class precision at
1 cycle/row PE throughput):
  mm1:  hT[h, s]  = sum_d W1[d, h] * qT[d, s]  (lhsT = W1 tile, rhs = qT tile)
        tanh fused on ScalarE with per-partition bias = (k @ W2)[h]
  mm2:  sc[s, v]  = sum_h hT[h, s] * V[h, v]   (lhsT = hT tile,  rhs = V tile)
        -> natural [S, v] layout, softmax over the free dim
  softmax: exp on ScalarE with accum_out giving per-row sums; reciprocal +
        scale on VectorE; weights DMA'd out in natural layout.
  values: p = w * q (VectorE), then partition-reduce via PE matmul with a
        ones[128,1] stationary vector (bf16: f32r disallows M=1/N=1),
        accumulated in PSUM across all S.

DMA: startup weight loads are striped across the three DGE descriptor
streams (sync HWDGE: q-transposed tiles; scalar HWDGE: W1; gpsimd SWDGE:
kT/W2/V) so the first matmul isn't queued behind 10 MB of weights.
"""

import numpy as np
import ml_dtypes
from contextlib import ExitStack

try:
    import concourse.bass as bass  # noqa: F401
except ImportError:  # pragma: no cover - defensive for fresh grading dirs
    import sys

    sys.path.insert(0, "/root/.axon_site/_ro/trn_rl_repo")

import concourse.bass as bass
import concourse.tile as tile
from concourse import bacc, mybir
from concourse.bass_utils import run_bass_kernel_spmd

P = 128
B, S, D = 8, 4096, 1024
ND = D // P           # 8 blocks of 128 along D/H/v
SGRP = 512            # seq columns processed per mm1 group
F32 = mybir.dt.float32
F32R = mybir.dt.float32r
BF16 = mybir.dt.bfloat16
TANH = mybir.ActivationFunctionType.Tanh
EXP = mybir.ActivationFunctionType.Exp


def build_program(s_len=S):
    """Build the per-core Bass program (same program on all 8 cores)."""
    nsg = s_len // SGRP
    nsb = s_len // P

    nc = bacc.Bacc("TRN2", target_bir_lowering=False, debug=False)

    qT = nc.dram_tensor("qT", [D, s_len], F32R, kind="ExternalInput")
    qn = nc.dram_tensor("qn", [s_len, D], BF16, kind="ExternalInput")
    w1 = nc.dram_tensor("w1", [D, D], F32R, kind="ExternalInput")
    vv = nc.dram_tensor("vv", [D, D], F32R, kind="ExternalInput")
    w2 = nc.dram_tensor("w2", [D, D], BF16, kind="ExternalInput")
    kT = nc.dram_tensor("kT", [P, ND], BF16, kind="ExternalInput")
    wout = nc.dram_tensor("wout", [s_len, D], F32, kind="ExternalOutput")
    vout = nc.dram_tensor("vout", [1, D], F32, kind="ExternalOutput")

    with tile.TileContext(nc) as tc, ExitStack() as ctx:
        const = ctx.enter_context(tc.tile_pool(name="const", bufs=1))
        w1p = ctx.enter_context(tc.tile_pool(name="w1p", bufs=1))
        vp = ctx.enter_context(tc.tile_pool(name="vp", bufs=1))
        w2p = ctx.enter_context(tc.tile_pool(name="w2p", bufs=1))
        qtsp = ctx.enter_context(tc.tile_pool(name="qtsp", bufs=16))
        htsp = ctx.enter_context(tc.tile_pool(name="htsp", bufs=16))
        expp = ctx.enter_context(tc.tile_pool(name="expp", bufs=3))
        wsbp = ctx.enter_context(tc.tile_pool(name="wsbp", bufs=3))
        qnp = ctx.enter_context(tc.tile_pool(name="qnp", bufs=4))
        pp = ctx.enter_context(tc.tile_pool(name="pp", bufs=3))
        smp = ctx.enter_context(tc.tile_pool(name="smp", bufs=4))
        ps1 = ctx.enter_context(tc.tile_pool(name="ps1", bufs=2, space="PSUM"))
        ps2 = ctx.enter_context(tc.tile_pool(name="ps2", bufs=2, space="PSUM"))
        psv = ctx.enter_context(tc.tile_pool(name="psv", bufs=1, space="PSUM"))

        ones_t = const.tile([P, 1], BF16, name="ones_t")
        nc.vector.memset(ones_t[:], 1.0)

        # sync (HWDGE) stream: first group of qT tiles before anything else
        qts_all = {}
        for d in range(ND):
            qt = qtsp.tile([P, SGRP], F32R, name=f"qts_0_{d}", tag="qts")
            nc.sync.dma_start(qt[:], qT.ap()[d * P:(d + 1) * P, 0:SGRP])
            qts_all[(0, d)] = qt

        # scalar (HWDGE) stream: kT, W2 rows (h_k chain), W1 rows, then V rows
        kt_sb = const.tile([P, ND], BF16, name="kt_sb")
        nc.scalar.dma_start(kt_sb[:], kT.ap())
        w2_t = []
        for i in range(ND):
            w2t = w2p.tile([P, D], BF16, name=f"w2t_{i}", tag=f"w2_{i}")
            nc.scalar.dma_start(w2t[:], w2.ap()[i * P:(i + 1) * P, :])
            w2_t.append(w2t)

        # W1 rows follow W2 on the scalar stream
        w1_t = []
        for i in range(ND):
            w1t = w1p.tile([P, D], F32R, name=f"w1t_{i}", tag=f"w1_{i}")
            nc.scalar.dma_start(w1t[:], w1.ap()[i * P:(i + 1) * P, :])
            w1_t.append(w1t)

        # h_k = k @ W2 as a [128, ND] column stack (h on partitions)
        hk_ps = ps1.tile([P, ND], F32, name="hk_ps", tag="m1")
        for h in range(ND):
            for d in range(ND):
                nc.tensor.matmul(
                    hk_ps[:, h:h + 1],
                    w2_t[d][:, h * P:(h + 1) * P],
                    kt_sb[:, d:d + 1],
                    start=(d == 0), stop=(d == ND - 1),
                )
        hk_sb = const.tile([P, ND], F32, name="hk_sb")
        nc.vector.tensor_copy(hk_sb[:], hk_ps[:])

        # V rows ride the scalar stream last (needed from first mm2 on)
        v_t = []
        for i in range(ND):
            vt = vp.tile([P, D], F32R, name=f"vt_{i}", tag=f"v_{i}")
            nc.scalar.dma_start(vt[:], vv.ap()[i * P:(i + 1) * P, :])
            v_t.append(vt)

        vps0 = psv.tile([1, 512], F32, name="vps0", tag="vps0")
        vps1 = psv.tile([1, 512], F32, name="vps1", tag="vps1")
        vps = [vps0, vps1]

        for g in range(nsg):
            qts = []
            for d in range(ND):
                if (g, d) in qts_all:
                    qts.append(qts_all[(g, d)])
                    continue
                qt = qtsp.tile([P, SGRP], F32R, name=f"qts_{g}_{d}", tag="qts")
                nc.sync.dma_start(
                    qt[:], qT.ap()[d * P:(d + 1) * P, g * SGRP:(g + 1) * SGRP]
                )
                qts.append(qt)

            hts = []
            for h in range(ND):
                m1 = ps1.tile([P, SGRP], F32, name=f"m1_{g}_{h}", tag="m1")
                for d in range(ND):
                    nc.tensor.matmul(
                        m1[:], w1_t[d][:, h * P:(h + 1) * P], qts[d][:],
                        start=(d == 0), stop=(d == ND - 1),
                    )
                ht = htsp.tile([P, SGRP], F32R, name=f"ht_{g}_{h}", tag="ht")
                nc.scalar.activation(ht[:], m1[:], TANH, bias=hk_sb[:, h:h + 1])
                hts.append(ht)

            for sb in range(SGRP // P):
                s_blk = g * (SGRP // P) + sb
                m2 = ps2.tile([P, D], F32, name=f"m2_{s_blk}", tag="m2")
                for h in range(ND):
                    lhs = hts[h][:, sb * P:(sb + 1) * P]
                    for vh in range(2):
                        nc.tensor.matmul(
                            m2[:, vh * 512:(vh + 1) * 512], lhs,
                            v_t[h][:, vh * 512:(vh + 1) * 512],
                            start=(h == 0), stop=(h == ND - 1),
                        )
                exp_t = expp.tile([P, D], F32, name=f"exp_{s_blk}", tag="exp")
                sum_t = smp.tile([P, 1], F32, name=f"sum_{s_blk}", tag="sum")
                nc.scalar.activation(exp_t[:], m2[:], EXP, accum_out=sum_t[:])
                rec_t = smp.tile([P, 1], F32, name=f"rec_{s_blk}", tag="rec")
                nc.vector.reciprocal(rec_t[:], sum_t[:])
                w_t = wsbp.tile([P, D], F32, name=f"w_{s_blk}", tag="w")
                nc.vector.tensor_scalar_mul(w_t[:], exp_t[:], rec_t[:])
                nc.gpsimd.dma_start(wout.ap()[s_blk * P:(s_blk + 1) * P, :], w_t[:])

                qn_t = qnp.tile([P, D], BF16, name=f"qn_{s_blk}", tag="qn")
                nc.sync.dma_start(qn_t[:], qn.ap()[s_blk * P:(s_blk + 1) * P, :])
                p_t = pp.tile([P, D], BF16, name=f"p_{s_blk}", tag="p")
                nc.vector.tensor_mul(p_t[:], w_t[:], qn_t[:])
                for vh in range(2):
                    nc.tensor.matmul(
                        vps[vh][:], ones_t[:], p_t[:, vh * 512:(vh + 1) * 512],
                        start=(s_blk == 0), stop=(s_blk == nsb - 1),
                        skip_group_check=True,
                    )

        vals_sb = const.tile([1, D], F32, name="vals_sb")
        nc.vector.tensor_copy(vals_sb[:, 0:512], vps0[:])
        nc.vector.tensor_copy(vals_sb[:, 512:1024], vps1[:])
        nc.gpsimd.dma_start(vout.ap(), vals_sb[:])

    nc.compile()
    return nc


def make_in_maps(q, k, W1, W2, V, s_len=S):
    q = np.asarray(q, dtype=np.float32)
    k = np.asarray(k, dtype=np.float32)
    W1 = np.ascontiguousarray(np.asarray(W1, dtype=np.float32))
    W2 = np.ascontiguousarray(np.asarray(W2, dtype=np.float32))
    V = np.ascontiguousarray(np.asarray(V, dtype=np.float32))
    w2_bf = W2.astype(ml_dtypes.bfloat16)
    in_maps = []
    for c in range(B):
        qc = q[c, :s_len, :]
        in_maps.append({
            "qT": np.ascontiguousarray(qc.T),
            "qn": qc.astype(ml_dtypes.bfloat16),
            "w1": W1,
            "vv": V,
            "w2": w2_bf,
            "kT": np.ascontiguousarray(k[c].reshape(ND, P).T).astype(ml_dtypes.bfloat16),
        })
    return in_maps


_cached = {}


def run(q, k, W1, W2, V, trace=False):
    if "nc" not in _cached:
        _cached["nc"] = build_program(S)
    nc = _cached["nc"]
    in_maps = make_in_maps(q, k, W1, W2, V)
    res = run_bass_kernel_spmd(nc, in_maps, core_ids=list(range(B)), trace=trace)
    values = np.stack([res.results[c]["vout"][0] for c in range(B)])
    weights = np.stack([res.results[c]["wout"] for c in range(B)])
    return values, weights, res


def kernel(q, k, W1, W2, V):
    values, weights, _ = run(q, k, W1, W2, V)
    return values, weights


# revision 7
# speedup vs baseline: 1.0751x; 1.0651x over previous
"""Trainium2 Bass kernel for nn_Attention_45414984188085.

Reference (per batch b):
    h   = tanh(q[b] @ W1 + k[b] @ W2)          # [S, H]
    sc  = h @ V                                # [S, H]
    w   = softmax(sc, axis=-1)                 # softmax over feature dim
    val = sum_s w[s, :] * q[b, s, :]           # [D]
    returns (values [B, D], weights [B, S, D])

Sharding: data-parallel over batch B=8 across the 8 NeuronCores; each core
computes its batch fully (no collectives).

Per-core dataflow (big matmuls in float32r - fp32-class precision at
1 cycle/row PE throughput):
  mm1:  hT[h, s]  = sum_d W1[d, h] * qT[d, s]  (lhsT = W1 tile, rhs = qT tile)
        tanh fused on ScalarE with per-partition bias = (k @ W2)[h]
  mm2:  sc[s, v]  = sum_h hT[h, s] * V[h, v]   (lhsT = hT tile,  rhs = V tile)
        -> natural [S, v] layout, softmax over the free dim
  softmax: exp on ScalarE with accum_out giving per-row sums; reciprocal +
        scale on VectorE; weights DMA'd out in natural layout.
  values: p = w * q (VectorE), then partition-reduce via PE matmul with a
        ones[128,1] stationary vector (bf16: f32r disallows M=1/N=1),
        accumulated in PSUM across all S.

DMA: startup weight loads are striped across the three DGE descriptor
streams (sync HWDGE: q-transposed tiles; scalar HWDGE: W1; gpsimd SWDGE:
kT/W2/V) so the first matmul isn't queued behind 10 MB of weights.
"""

import numpy as np
import ml_dtypes
from contextlib import ExitStack

try:
    import concourse.bass as bass  # noqa: F401
except ImportError:  # pragma: no cover - defensive for fresh grading dirs
    import sys

    sys.path.insert(0, "/root/.axon_site/_ro/trn_rl_repo")

import concourse.bass as bass
import concourse.tile as tile
from concourse import bacc, mybir
from concourse.bass_utils import run_bass_kernel_spmd

P = 128
B, S, D = 8, 4096, 1024
ND = D // P           # 8 blocks of 128 along D/H/v
SGRP = 512            # seq columns processed per mm1 group
F32 = mybir.dt.float32
F32R = mybir.dt.float32r
BF16 = mybir.dt.bfloat16
TANH = mybir.ActivationFunctionType.Tanh
EXP = mybir.ActivationFunctionType.Exp


MM_DT = BF16      # dtype for the two big matmuls (F32R or BF16)


def build_program(s_len=S, mm_dt=None):
    """Build the per-core Bass program (same program on all 8 cores)."""
    if mm_dt is None:
        mm_dt = MM_DT
    nsg = s_len // SGRP
    nsb = s_len // P

    nc = bacc.Bacc("TRN2", target_bir_lowering=False, debug=False)

    qT = nc.dram_tensor("qT", [D, s_len], mm_dt, kind="ExternalInput")
    qn = nc.dram_tensor("qn", [s_len, D], BF16, kind="ExternalInput")
    w1 = nc.dram_tensor("w1", [D, D], mm_dt, kind="ExternalInput")
    vv = nc.dram_tensor("vv", [D, D], mm_dt, kind="ExternalInput")
    w2 = nc.dram_tensor("w2", [D, D], BF16, kind="ExternalInput")
    kT = nc.dram_tensor("kT", [P, ND], BF16, kind="ExternalInput")
    wout = nc.dram_tensor("wout", [s_len, D], F32, kind="ExternalOutput")
    vout = nc.dram_tensor("vout", [1, D], F32, kind="ExternalOutput")

    with tile.TileContext(nc) as tc, ExitStack() as ctx:
        const = ctx.enter_context(tc.tile_pool(name="const", bufs=1))
        w1p = ctx.enter_context(tc.tile_pool(name="w1p", bufs=1))
        vp = ctx.enter_context(tc.tile_pool(name="vp", bufs=1))
        w2p = ctx.enter_context(tc.tile_pool(name="w2p", bufs=1))
        qtsp = ctx.enter_context(tc.tile_pool(name="qtsp", bufs=16))
        htsp = ctx.enter_context(tc.tile_pool(name="htsp", bufs=16))
        expp = ctx.enter_context(tc.tile_pool(name="expp", bufs=3))
        wsbp = ctx.enter_context(tc.tile_pool(name="wsbp", bufs=3))
        qnp = ctx.enter_context(tc.tile_pool(name="qnp", bufs=4))
        pp = ctx.enter_context(tc.tile_pool(name="pp", bufs=3))
        smp = ctx.enter_context(tc.tile_pool(name="smp", bufs=4))
        ps1 = ctx.enter_context(tc.tile_pool(name="ps1", bufs=2, space="PSUM"))
        ps2 = ctx.enter_context(tc.tile_pool(name="ps2", bufs=2, space="PSUM"))
        psv = ctx.enter_context(tc.tile_pool(name="psv", bufs=1, space="PSUM"))

        ones_t = const.tile([P, 1], BF16, name="ones_t")
        nc.vector.memset(ones_t[:], 1.0)

        # sync (HWDGE) stream: first group of qT tiles before anything else
        qts_all = {}
        for d in range(ND):
            qt = qtsp.tile([P, SGRP], mm_dt, name=f"qts_0_{d}", tag="qts")
            nc.sync.dma_start(qt[:], qT.ap()[d * P:(d + 1) * P, 0:SGRP])
            qts_all[(0, d)] = qt

        # scalar (HWDGE) stream: kT, W2 rows (h_k chain), W1 rows, then V rows
        kt_sb = const.tile([P, ND], BF16, name="kt_sb")
        nc.scalar.dma_start(kt_sb[:], kT.ap())
        w2_t = []
        for i in range(ND):
            w2t = w2p.tile([P, D], BF16, name=f"w2t_{i}", tag=f"w2_{i}")
            nc.scalar.dma_start(w2t[:], w2.ap()[i * P:(i + 1) * P, :])
            w2_t.append(w2t)

        # W1 rows follow W2 on the scalar stream
        w1_t = []
        for i in range(ND):
            w1t = w1p.tile([P, D], mm_dt, name=f"w1t_{i}", tag=f"w1_{i}")
            nc.scalar.dma_start(w1t[:], w1.ap()[i * P:(i + 1) * P, :])
            w1_t.append(w1t)

        # h_k = k @ W2 as a [128, ND] column stack (h on partitions)
        hk_ps = ps1.tile([P, ND], F32, name="hk_ps", tag="m1")
        for h in range(ND):
            for d in range(ND):
                nc.tensor.matmul(
                    hk_ps[:, h:h + 1],
                    w2_t[d][:, h * P:(h + 1) * P],
                    kt_sb[:, d:d + 1],
                    start=(d == 0), stop=(d == ND - 1),
                )
        hk_sb = const.tile([P, ND], F32, name="hk_sb")
        nc.vector.tensor_copy(hk_sb[:], hk_ps[:])

        # V rows ride the scalar stream last (needed from first mm2 on)
        v_t = []
        for i in range(ND):
            vt = vp.tile([P, D], mm_dt, name=f"vt_{i}", tag=f"v_{i}")
            nc.scalar.dma_start(vt[:], vv.ap()[i * P:(i + 1) * P, :])
            v_t.append(vt)

        vps0 = psv.tile([1, 512], F32, name="vps0", tag="vps0")
        vps1 = psv.tile([1, 512], F32, name="vps1", tag="vps1")
        vps = [vps0, vps1]

        for g in range(nsg):
            qts = []
            for d in range(ND):
                if (g, d) in qts_all:
                    qts.append(qts_all[(g, d)])
                    continue
                qt = qtsp.tile([P, SGRP], mm_dt, name=f"qts_{g}_{d}", tag="qts")
                nc.sync.dma_start(
                    qt[:], qT.ap()[d * P:(d + 1) * P, g * SGRP:(g + 1) * SGRP]
                )
                qts.append(qt)

            hts = []
            for h in range(ND):
                m1 = ps1.tile([P, SGRP], F32, name=f"m1_{g}_{h}", tag="m1")
                for d in range(ND):
                    nc.tensor.matmul(
                        m1[:], w1_t[d][:, h * P:(h + 1) * P], qts[d][:],
                        start=(d == 0), stop=(d == ND - 1),
                    )
                ht = htsp.tile([P, SGRP], mm_dt, name=f"ht_{g}_{h}", tag="ht")
                nc.scalar.activation(ht[:], m1[:], TANH, bias=hk_sb[:, h:h + 1])
                hts.append(ht)

            for sb in range(SGRP // P):
                s_blk = g * (SGRP // P) + sb
                m2 = ps2.tile([P, D], F32, name=f"m2_{s_blk}", tag="m2")
                for h in range(ND):
                    lhs = hts[h][:, sb * P:(sb + 1) * P]
                    for vh in range(2):
                        nc.tensor.matmul(
                            m2[:, vh * 512:(vh + 1) * 512], lhs,
                            v_t[h][:, vh * 512:(vh + 1) * 512],
                            start=(h == 0), stop=(h == ND - 1),
                        )
                exp_t = expp.tile([P, D], F32, name=f"exp_{s_blk}", tag="exp")
                sum_t = smp.tile([P, 1], F32, name=f"sum_{s_blk}", tag="sum")
                nc.scalar.activation(exp_t[:], m2[:], EXP, accum_out=sum_t[:])
                rec_t = smp.tile([P, 1], F32, name=f"rec_{s_blk}", tag="rec")
                nc.vector.reciprocal(rec_t[:], sum_t[:])
                w_t = wsbp.tile([P, D], F32, name=f"w_{s_blk}", tag="w")
                nc.vector.tensor_scalar_mul(w_t[:], exp_t[:], rec_t[:])
                nc.gpsimd.dma_start(wout.ap()[s_blk * P:(s_blk + 1) * P, :], w_t[:])

                qn_t = qnp.tile([P, D], BF16, name=f"qn_{s_blk}", tag="qn")
                nc.sync.dma_start(qn_t[:], qn.ap()[s_blk * P:(s_blk + 1) * P, :])
                p_t = pp.tile([P, D], BF16, name=f"p_{s_blk}", tag="p")
                nc.vector.tensor_mul(p_t[:], w_t[:], qn_t[:])
                for vh in range(2):
                    nc.tensor.matmul(
                        vps[vh][:], ones_t[:], p_t[:, vh * 512:(vh + 1) * 512],
                        start=(s_blk == 0), stop=(s_blk == nsb - 1),
                        skip_group_check=True,
                    )

        vals_sb = const.tile([1, D], F32, name="vals_sb")
        nc.vector.tensor_copy(vals_sb[:, 0:512], vps0[:])
        nc.vector.tensor_copy(vals_sb[:, 512:1024], vps1[:])
        nc.gpsimd.dma_start(vout.ap(), vals_sb[:])

    nc.compile()
    return nc


def make_in_maps(q, k, W1, W2, V, s_len=S, mm_dt=None):
    if mm_dt is None:
        mm_dt = MM_DT
    np_mm = np.float32 if mm_dt == F32R else ml_dtypes.bfloat16
    q = np.asarray(q, dtype=np.float32)
    k = np.asarray(k, dtype=np.float32)
    W1 = np.ascontiguousarray(np.asarray(W1, dtype=np.float32))
    W2 = np.ascontiguousarray(np.asarray(W2, dtype=np.float32))
    V = np.ascontiguousarray(np.asarray(V, dtype=np.float32))
    w2_bf = W2.astype(ml_dtypes.bfloat16)
    in_maps = []
    for c in range(B):
        qc = q[c, :s_len, :]
        in_maps.append({
            "qT": np.ascontiguousarray(qc.T).astype(np_mm, copy=False),
            "qn": qc.astype(ml_dtypes.bfloat16),
            "w1": W1.astype(np_mm, copy=False),
            "vv": V.astype(np_mm, copy=False),
            "w2": w2_bf,
            "kT": np.ascontiguousarray(k[c].reshape(ND, P).T).astype(ml_dtypes.bfloat16),
        })
    return in_maps


_cached = {}


def run(q, k, W1, W2, V, trace=False):
    if "nc" not in _cached:
        _cached["nc"] = build_program(S)
    nc = _cached["nc"]
    in_maps = make_in_maps(q, k, W1, W2, V)
    res = run_bass_kernel_spmd(nc, in_maps, core_ids=list(range(B)), trace=trace)
    values = np.stack([res.results[c]["vout"][0] for c in range(B)])
    weights = np.stack([res.results[c]["wout"] for c in range(B)])
    return values, weights, res


def kernel(q, k, W1, W2, V):
    values, weights, _ = run(q, k, W1, W2, V)
    return values, weights


# revision 9
# speedup vs baseline: 1.0796x; 1.0042x over previous
"""Trainium2 Bass kernel for nn_Attention_45414984188085.

Reference (per batch b):
    h   = tanh(q[b] @ W1 + k[b] @ W2)          # [S, H]
    sc  = h @ V                                # [S, H]
    w   = softmax(sc, axis=-1)                 # softmax over feature dim
    val = sum_s w[s, :] * q[b, s, :]           # [D]
    returns (values [B, D], weights [B, S, D])

Sharding: data-parallel over batch B=8 across the 8 NeuronCores; each core
computes its batch fully (no collectives).

Per-core dataflow (big matmuls in float32r - fp32-class precision at
1 cycle/row PE throughput):
  mm1:  hT[h, s]  = sum_d W1[d, h] * qT[d, s]  (lhsT = W1 tile, rhs = qT tile)
        tanh fused on ScalarE with per-partition bias = (k @ W2)[h]
  mm2:  sc[s, v]  = sum_h hT[h, s] * V[h, v]   (lhsT = hT tile,  rhs = V tile)
        -> natural [S, v] layout, softmax over the free dim
  softmax: exp on ScalarE with accum_out giving per-row sums; reciprocal +
        scale on VectorE; weights DMA'd out in natural layout.
  values: p = w * q (VectorE), then partition-reduce via PE matmul with a
        ones[128,1] stationary vector (bf16: f32r disallows M=1/N=1),
        accumulated in PSUM across all S.

DMA: startup weight loads are striped across the three DGE descriptor
streams (sync HWDGE: q-transposed tiles; scalar HWDGE: W1; gpsimd SWDGE:
kT/W2/V) so the first matmul isn't queued behind 10 MB of weights.
"""

import numpy as np
import ml_dtypes
from contextlib import ExitStack

try:
    import concourse.bass as bass  # noqa: F401
except ImportError:  # pragma: no cover - defensive for fresh grading dirs
    import sys

    sys.path.insert(0, "/root/.axon_site/_ro/trn_rl_repo")

import concourse.bass as bass
import concourse.tile as tile
from concourse import bacc, mybir
from concourse.bass_utils import run_bass_kernel_spmd

P = 128
B, S, D = 8, 4096, 1024
ND = D // P           # 8 blocks of 128 along D/H/v
SGRP = 512            # seq columns processed per mm1 group
F32 = mybir.dt.float32
F32R = mybir.dt.float32r
BF16 = mybir.dt.bfloat16
TANH = mybir.ActivationFunctionType.Tanh
EXP = mybir.ActivationFunctionType.Exp


MM_DT = BF16      # dtype for the two big matmuls (F32R or BF16)


def build_program(s_len=S, mm_dt=None):
    """Build the per-core Bass program (same program on all 8 cores)."""
    if mm_dt is None:
        mm_dt = MM_DT
    nsg = s_len // SGRP
    nsb = s_len // P

    nc = bacc.Bacc("TRN2", target_bir_lowering=False, debug=False)

    qT = nc.dram_tensor("qT", [D, s_len], mm_dt, kind="ExternalInput")
    qn = nc.dram_tensor("qn", [s_len, D], BF16, kind="ExternalInput")
    w1 = nc.dram_tensor("w1", [D, D], mm_dt, kind="ExternalInput")
    vv = nc.dram_tensor("vv", [D, D], mm_dt, kind="ExternalInput")
    w2 = nc.dram_tensor("w2", [D, D], BF16, kind="ExternalInput")
    kT = nc.dram_tensor("kT", [P, ND], BF16, kind="ExternalInput")
    wout = nc.dram_tensor("wout", [s_len, D], F32, kind="ExternalOutput")
    vout = nc.dram_tensor("vout", [1, D], F32, kind="ExternalOutput")

    with tile.TileContext(nc) as tc, ExitStack() as ctx:
        const = ctx.enter_context(tc.tile_pool(name="const", bufs=1))
        w1p = ctx.enter_context(tc.tile_pool(name="w1p", bufs=1))
        vp = ctx.enter_context(tc.tile_pool(name="vp", bufs=1))
        w2p = ctx.enter_context(tc.tile_pool(name="w2p", bufs=1))
        qtsp = ctx.enter_context(tc.tile_pool(name="qtsp", bufs=3))
        htsp = ctx.enter_context(tc.tile_pool(name="htsp", bufs=16))
        expp = ctx.enter_context(tc.tile_pool(name="expp", bufs=3))
        wsbp = ctx.enter_context(tc.tile_pool(name="wsbp", bufs=3))
        qnp = ctx.enter_context(tc.tile_pool(name="qnp", bufs=2))
        pp = ctx.enter_context(tc.tile_pool(name="pp", bufs=3))
        smp = ctx.enter_context(tc.tile_pool(name="smp", bufs=4))
        ps1 = ctx.enter_context(tc.tile_pool(name="ps1", bufs=2, space="PSUM"))
        ps2 = ctx.enter_context(tc.tile_pool(name="ps2", bufs=2, space="PSUM"))
        psv = ctx.enter_context(tc.tile_pool(name="psv", bufs=1, space="PSUM"))

        ones_t = const.tile([P, 1], BF16, name="ones_t")
        nc.vector.memset(ones_t[:], 1.0)

        # sync (HWDGE) stream: first group of qT tiles before anything else
        def load_qts(g):
            qf = qtsp.tile([P, ND * SGRP], mm_dt, name=f"qts_{g}", tag="qts")
            nc.sync.dma_start(
                qf[:].rearrange("p (r s) -> p r s", r=ND),
                qT.ap()[:, g * SGRP:(g + 1) * SGRP]
                    .rearrange("(r p) s -> p r s", p=P),
            )
            return [qf[:, d * SGRP:(d + 1) * SGRP] for d in range(ND)]

        qts_all = {0: load_qts(0)}

        # scalar (HWDGE) stream: kT, W2, W1, then V - each as ONE strided DMA
        # (row-block r, column c of the [D, D] matrix lands at flat column
        # r * D + c; block r lives on partitions p = row mod 128)
        kt_sb = const.tile([P, ND], BF16, name="kt_sb")
        nc.scalar.dma_start(kt_sb[:], kT.ap())
        w2_fl = w2p.tile([P, ND * D], BF16, name="w2_fl")
        nc.scalar.dma_start(
            w2_fl[:].rearrange("p (r c) -> p r c", r=ND),
            w2.ap().rearrange("(r p) c -> p r c", p=P),
        )
        w1_fl = w1p.tile([P, ND * D], mm_dt, name="w1_fl")
        nc.scalar.dma_start(
            w1_fl[:].rearrange("p (r c) -> p r c", r=ND),
            w1.ap().rearrange("(r p) c -> p r c", p=P),
        )
        w2_t = [w2_fl[:, i * D:(i + 1) * D] for i in range(ND)]
        w1_t = [w1_fl[:, i * D:(i + 1) * D] for i in range(ND)]

        # h_k = k @ W2 as a [128, ND] column stack (h on partitions)
        hk_ps = ps1.tile([P, ND], F32, name="hk_ps", tag="m1")
        for h in range(ND):
            for d in range(ND):
                nc.tensor.matmul(
                    hk_ps[:, h:h + 1],
                    w2_t[d][:, h * P:(h + 1) * P],
                    kt_sb[:, d:d + 1],
                    start=(d == 0), stop=(d == ND - 1),
                )
        hk_sb = const.tile([P, ND], F32, name="hk_sb")
        nc.vector.tensor_copy(hk_sb[:], hk_ps[:])

        # V rides the scalar stream last (needed from first mm2 on)
        v_fl = vp.tile([P, ND * D], mm_dt, name="v_fl")
        nc.scalar.dma_start(
            v_fl[:].rearrange("p (r c) -> p r c", r=ND),
            vv.ap().rearrange("(r p) c -> p r c", p=P),
        )
        v_t = [v_fl[:, i * D:(i + 1) * D] for i in range(ND)]

        vps0 = psv.tile([1, 512], F32, name="vps0", tag="vps0")
        vps1 = psv.tile([1, 512], F32, name="vps1", tag="vps1")
        vps = [vps0, vps1]

        for g in range(nsg):
            qts = qts_all.pop(g) if g in qts_all else load_qts(g)

            hts = []
            for h in range(ND):
                m1 = ps1.tile([P, SGRP], F32, name=f"m1_{g}_{h}", tag="m1")
                for d in range(ND):
                    nc.tensor.matmul(
                        m1[:], w1_t[d][:, h * P:(h + 1) * P], qts[d][:],
                        start=(d == 0), stop=(d == ND - 1),
                    )
                ht = htsp.tile([P, SGRP], mm_dt, name=f"ht_{g}_{h}", tag="ht")
                nc.scalar.activation(ht[:], m1[:], TANH, bias=hk_sb[:, h:h + 1])
                hts.append(ht)

            qn_fl = qnp.tile([P, (SGRP // P) * D], BF16, name=f"qn_{g}", tag="qn")
            nc.sync.dma_start(
                qn_fl[:].rearrange("p (r c) -> p r c", r=SGRP // P),
                qn.ap()[g * SGRP:(g + 1) * SGRP, :]
                    .rearrange("(r p) c -> p r c", p=P),
            )
            for sb in range(SGRP // P):
                s_blk = g * (SGRP // P) + sb
                m2 = ps2.tile([P, D], F32, name=f"m2_{s_blk}", tag="m2")
                for h in range(ND):
                    lhs = hts[h][:, sb * P:(sb + 1) * P]
                    for vh in range(2):
                        nc.tensor.matmul(
                            m2[:, vh * 512:(vh + 1) * 512], lhs,
                            v_t[h][:, vh * 512:(vh + 1) * 512],
                            start=(h == 0), stop=(h == ND - 1),
                        )
                exp_t = expp.tile([P, D], F32, name=f"exp_{s_blk}", tag="exp")
                sum_t = smp.tile([P, 1], F32, name=f"sum_{s_blk}", tag="sum")
                nc.scalar.activation(exp_t[:], m2[:], EXP, accum_out=sum_t[:])
                rec_t = smp.tile([P, 1], F32, name=f"rec_{s_blk}", tag="rec")
                nc.vector.reciprocal(rec_t[:], sum_t[:])
                w_t = wsbp.tile([P, D], F32, name=f"w_{s_blk}", tag="w")
                nc.vector.tensor_scalar_mul(w_t[:], exp_t[:], rec_t[:])
                nc.gpsimd.dma_start(wout.ap()[s_blk * P:(s_blk + 1) * P, :], w_t[:])

                p_t = pp.tile([P, D], BF16, name=f"p_{s_blk}", tag="p")
                nc.vector.tensor_mul(p_t[:], w_t[:], qn_fl[:, sb * D:(sb + 1) * D])
                for vh in range(2):
                    nc.tensor.matmul(
                        vps[vh][:], ones_t[:], p_t[:, vh * 512:(vh + 1) * 512],
                        start=(s_blk == 0), stop=(s_blk == nsb - 1),
                        skip_group_check=True,
                    )

        vals_sb = const.tile([1, D], F32, name="vals_sb")
        nc.vector.tensor_copy(vals_sb[:, 0:512], vps0[:])
        nc.vector.tensor_copy(vals_sb[:, 512:1024], vps1[:])
        nc.gpsimd.dma_start(vout.ap(), vals_sb[:])

    nc.compile()
    return nc


def make_in_maps(q, k, W1, W2, V, s_len=S, mm_dt=None):
    if mm_dt is None:
        mm_dt = MM_DT
    np_mm = np.float32 if mm_dt == F32R else ml_dtypes.bfloat16
    q = np.asarray(q, dtype=np.float32)
    k = np.asarray(k, dtype=np.float32)
    W1 = np.ascontiguousarray(np.asarray(W1, dtype=np.float32))
    W2 = np.ascontiguousarray(np.asarray(W2, dtype=np.float32))
    V = np.ascontiguousarray(np.asarray(V, dtype=np.float32))
    w2_bf = W2.astype(ml_dtypes.bfloat16)
    in_maps = []
    for c in range(B):
        qc = q[c, :s_len, :]
        in_maps.append({
            "qT": np.ascontiguousarray(qc.T).astype(np_mm, copy=False),
            "qn": qc.astype(ml_dtypes.bfloat16),
            "w1": W1.astype(np_mm, copy=False),
            "vv": V.astype(np_mm, copy=False),
            "w2": w2_bf,
            "kT": np.ascontiguousarray(k[c].reshape(ND, P).T).astype(ml_dtypes.bfloat16),
        })
    return in_maps


_cached = {}


def run(q, k, W1, W2, V, trace=False):
    if "nc" not in _cached:
        _cached["nc"] = build_program(S)
    nc = _cached["nc"]
    in_maps = make_in_maps(q, k, W1, W2, V)
    res = run_bass_kernel_spmd(nc, in_maps, core_ids=list(range(B)), trace=trace)
    values = np.stack([res.results[c]["vout"][0] for c in range(B)])
    weights = np.stack([res.results[c]["wout"] for c in range(B)])
    return values, weights, res


def kernel(q, k, W1, W2, V):
    values, weights, _ = run(q, k, W1, W2, V)
    return values, weights


# revision 23
# speedup vs baseline: 1.0917x; 1.0112x over previous
"""Trainium2 Bass kernel for nn_Attention_45414984188085.

Reference (per batch b):
    h   = tanh(q[b] @ W1 + k[b] @ W2)          # [S, H]
    sc  = h @ V                                # [S, H]
    w   = softmax(sc, axis=-1)                 # softmax over feature dim
    val = sum_s w[s, :] * q[b, s, :]           # [D]
    returns (values [B, D], weights [B, S, D])

Sharding: data-parallel over batch B=8 across the 8 NeuronCores; each core
computes its batch fully (no collectives).

Per-core dataflow (big matmuls in bf16 with fp32 PSUM accumulation; the
MM_DT switch below selects float32r instead for ~2x tighter accuracy at
~8% more time):
  mm1:  hT[h, s]  = sum_d W1[d, h] * qT[d, s]  (lhsT = W1 tile, rhs = qT tile)
        tanh fused on ScalarE with per-partition bias = (k @ W2)[h]
  mm2:  sc[s, v]  = sum_h hT[h, s] * V[h, v]   (lhsT = hT tile,  rhs = V tile)
        -> natural [S, v] layout, softmax over the free dim
  softmax: exp on ScalarE with accum_out giving per-row sums; reciprocal +
        scale on VectorE; weights DMA'd out in natural layout.
  values: p = w * q (VectorE), then partition-reduce via PE matmul with a
        ones[128,1] stationary vector (bf16: f32r disallows M=1/N=1),
        accumulated in PSUM across all S.

DMA: per-HWDGE-queue bandwidth is roughly half the core's HBM rate, so
startup weight loads are split row-wise across the sync and scalar HWDGE
queues, the q streams ride sync, and the (compute-gated) weights stores
ride gpsimd's SWDGE so they never head-of-line-block a load queue.
"""

import numpy as np
import ml_dtypes
from contextlib import ExitStack

try:
    import concourse.bass as bass  # noqa: F401
except ImportError:  # pragma: no cover - defensive for fresh grading dirs
    import sys

    sys.path.insert(0, "/root/.axon_site/_ro/trn_rl_repo")

import concourse.bass as bass
import concourse.tile as tile
from concourse import bacc, mybir
from concourse.bass_utils import run_bass_kernel_spmd

P = 128
B, S, D = 8, 4096, 1024
ND = D // P           # 8 blocks of 128 along D/H/v
F32 = mybir.dt.float32
F32R = mybir.dt.float32r
BF16 = mybir.dt.bfloat16
TANH = mybir.ActivationFunctionType.Tanh
EXP = mybir.ActivationFunctionType.Exp


MM_DT = BF16      # dtype for the two big matmuls (F32R or BF16)


def build_program(s_len=S, mm_dt=None):
    """Build the per-core Bass program (same program on all 8 cores)."""
    if mm_dt is None:
        mm_dt = MM_DT
    groups = [512] * (s_len // 512) if s_len % 512 == 0 else [s_len]
    nsb = s_len // P

    nc = bacc.Bacc("TRN2", target_bir_lowering=False, debug=False)

    qT = nc.dram_tensor("qT", [D, s_len], mm_dt, kind="ExternalInput")
    qn = nc.dram_tensor("qn", [s_len, D], BF16, kind="ExternalInput")
    w1 = nc.dram_tensor("w1", [D, D], mm_dt, kind="ExternalInput")
    vv = nc.dram_tensor("vv", [D, D], mm_dt, kind="ExternalInput")
    w2 = nc.dram_tensor("w2", [D, D], BF16, kind="ExternalInput")
    kT = nc.dram_tensor("kT", [P, ND], BF16, kind="ExternalInput")
    wout = nc.dram_tensor("wout", [s_len, D], F32, kind="ExternalOutput")
    vout = nc.dram_tensor("vout", [1, D], F32, kind="ExternalOutput")

    with tile.TileContext(nc) as tc, ExitStack() as ctx:
        const = ctx.enter_context(tc.tile_pool(name="const", bufs=1))
        w1p = ctx.enter_context(tc.tile_pool(name="w1p", bufs=1))
        vp = ctx.enter_context(tc.tile_pool(name="vp", bufs=1))
        w2p = ctx.enter_context(tc.tile_pool(name="w2p", bufs=1))
        qtsp = ctx.enter_context(tc.tile_pool(name="qtsp", bufs=3))
        htsp = ctx.enter_context(tc.tile_pool(name="htsp", bufs=16))
        expp = ctx.enter_context(tc.tile_pool(name="expp", bufs=3))
        wsbp = ctx.enter_context(tc.tile_pool(name="wsbp", bufs=3))
        qnp = ctx.enter_context(tc.tile_pool(name="qnp", bufs=2))
        pp = ctx.enter_context(tc.tile_pool(name="pp", bufs=3))
        smp = ctx.enter_context(tc.tile_pool(name="smp", bufs=4))
        ps1 = ctx.enter_context(tc.tile_pool(name="ps1", bufs=2, space="PSUM"))
        ps2 = ctx.enter_context(tc.tile_pool(name="ps2", bufs=2, space="PSUM"))
        psv = ctx.enter_context(tc.tile_pool(name="psv", bufs=1, space="PSUM"))

        ones_t = const.tile([P, 1], BF16, name="ones_t")
        nc.vector.memset(ones_t[:], 1.0)

        # Startup loads: per-HWDGE-queue bandwidth is ~half the core's HBM
        # rate, so every big weight load is split row-wise across BOTH
        # queues (sync + scalar) to halve its latency.
        HALF = ND // 2
        kt_sb = const.tile([P, ND], BF16, name="kt_sb")
        nc.sync.dma_start(kt_sb[:], kT.ap())

        def load_rows_split(name, pool, dram, dt):
            fl_a = pool.tile([P, HALF * D], dt, name=f"{name}_a")
            fl_b = pool.tile([P, HALF * D], dt, name=f"{name}_b")
            src_ap = dram.ap().rearrange("(r p) c -> p r c", p=P)
            nc.sync.dma_start(
                fl_a[:].rearrange("p (r c) -> p r c", r=HALF), src_ap[:, 0:HALF, :]
            )
            nc.scalar.dma_start(
                fl_b[:].rearrange("p (r c) -> p r c", r=HALF),
                src_ap[:, HALF:ND, :],
            )
            return ([fl_a[:, i * D:(i + 1) * D] for i in range(HALF)]
                    + [fl_b[:, i * D:(i + 1) * D] for i in range(HALF)])

        w2_t = load_rows_split("w2", w2p, w2, BF16)
        w1_t = load_rows_split("w1", w1p, w1, mm_dt)

        goff = [sum(groups[:i]) for i in range(len(groups))]

        def load_qts(g, eng=None):
            gcols = groups[g]
            qf = qtsp.tile([P, ND * gcols], mm_dt, name=f"qts_{g}", tag="qts",
                           padded_shape=[P, ND * 512])
            (eng or nc.sync).dma_start(
                qf[:].rearrange("p (r s) -> p r s", r=ND),
                qT.ap()[:, goff[g]:goff[g] + gcols]
                    .rearrange("(r p) s -> p r s", p=P),
            )
            return [qf[:, d * gcols:(d + 1) * gcols] for d in range(ND)]

        qts_all = {0: load_qts(0)}
        if len(groups) > 1:
            qts_all[1] = load_qts(1, eng=nc.scalar)

        # V split by column halves into two tiles across both queues
        # (mm2's vh-th matmul then only waits on half vh)
        vsrc = vv.ap().rearrange("(r p) c -> p r c", p=P)
        v_ha = vp.tile([P, ND * 512], mm_dt, name="v_ha")
        nc.sync.dma_start(
            v_ha[:].rearrange("p (r c) -> p r c", r=ND), vsrc[:, :, 0:512]
        )
        v_hb = vp.tile([P, ND * 512], mm_dt, name="v_hb")
        nc.scalar.dma_start(
            v_hb[:].rearrange("p (r c) -> p r c", r=ND), vsrc[:, :, 512:1024]
        )
        v_half = [v_ha, v_hb]

        # h_k = k @ W2 as a [128, ND] column stack (h on partitions)
        hk_ps = ps1.tile([P, ND], F32, name="hk_ps", tag="m1")
        for h in range(ND):
            for d in range(ND):
                nc.tensor.matmul(
                    hk_ps[:, h:h + 1],
                    w2_t[d][:, h * P:(h + 1) * P],
                    kt_sb[:, d:d + 1],
                    start=(d == 0), stop=(d == ND - 1),
                )
        hk_sb = const.tile([P, ND], F32, name="hk_sb")
        nc.vector.tensor_copy(hk_sb[:], hk_ps[:])


        vps0 = psv.tile([1, 512], F32, name="vps0", tag="vps0")
        vps1 = psv.tile([1, 512], F32, name="vps1", tag="vps1")
        vps = [vps0, vps1]

        for g in range(len(groups)):
            gc = groups[g]
            qts = qts_all.pop(g) if g in qts_all else load_qts(g)

            hts = []
            for h in range(ND):
                m1 = ps1.tile([P, gc], F32, name=f"m1_{g}_{h}", tag="m1",
                              padded_shape=[P, 512])
                for d in range(ND):
                    nc.tensor.matmul(
                        m1[:], w1_t[d][:, h * P:(h + 1) * P], qts[d][:],
                        start=(d == 0), stop=(d == ND - 1),
                    )
                ht = htsp.tile([P, gc], mm_dt, name=f"ht_{g}_{h}", tag="ht",
                               padded_shape=[P, 512])
                nc.scalar.activation(ht[:], m1[:], TANH, bias=hk_sb[:, h:h + 1])
                hts.append(ht)

            qn_fl = qnp.tile([P, (gc // P) * D], BF16, name=f"qn_{g}", tag="qn",
                             padded_shape=[P, (512 // P) * D])
            nc.sync.dma_start(
                qn_fl[:].rearrange("p (r c) -> p r c", r=gc // P),
                qn.ap()[goff[g]:goff[g] + gc, :]
                    .rearrange("(r p) c -> p r c", p=P),
            )
            for sb in range(gc // P):
                s_blk = goff[g] // P + sb
                m2 = ps2.tile([P, D], F32, name=f"m2_{s_blk}", tag="m2")
                for h in range(ND):
                    lhs = hts[h][:, sb * P:(sb + 1) * P]
                    for vh in range(2):
                        nc.tensor.matmul(
                            m2[:, vh * 512:(vh + 1) * 512], lhs,
                            v_half[vh][:, h * 512:(h + 1) * 512],
                            start=(h == 0), stop=(h == ND - 1),
                        )
                exp_t = expp.tile([P, D], F32, name=f"exp_{s_blk}", tag="exp")
                sum_t = smp.tile([P, 1], F32, name=f"sum_{s_blk}", tag="sum")
                nc.scalar.activation(exp_t[:], m2[:], EXP, accum_out=sum_t[:])
                rec_t = smp.tile([P, 1], F32, name=f"rec_{s_blk}", tag="rec")
                nc.vector.reciprocal(rec_t[:], sum_t[:])
                w_t = wsbp.tile([P, D], F32, name=f"w_{s_blk}", tag="w")
                nc.vector.tensor_scalar_mul(w_t[:], exp_t[:], rec_t[:])
                nc.gpsimd.dma_start(wout.ap()[s_blk * P:(s_blk + 1) * P, :], w_t[:])

                p_t = pp.tile([P, D], BF16, name=f"p_{s_blk}", tag="p")
                nc.vector.tensor_mul(p_t[:], w_t[:], qn_fl[:, sb * D:(sb + 1) * D])
                for vh in range(2):
                    nc.tensor.matmul(
                        vps[vh][:], ones_t[:], p_t[:, vh * 512:(vh + 1) * 512],
                        start=(s_blk == 0), stop=(s_blk == nsb - 1),
                        skip_group_check=True,
                    )

        vals_sb = const.tile([1, D], F32, name="vals_sb")
        nc.vector.tensor_copy(vals_sb[:, 0:512], vps0[:])
        nc.vector.tensor_copy(vals_sb[:, 512:1024], vps1[:])
        nc.gpsimd.dma_start(vout.ap(), vals_sb[:])

    nc.compile()
    return nc


def make_in_maps(q, k, W1, W2, V, s_len=S, mm_dt=None):
    if mm_dt is None:
        mm_dt = MM_DT
    np_mm = np.float32 if mm_dt == F32R else ml_dtypes.bfloat16
    q = np.asarray(q, dtype=np.float32)
    k = np.asarray(k, dtype=np.float32)
    W1 = np.ascontiguousarray(np.asarray(W1, dtype=np.float32))
    W2 = np.ascontiguousarray(np.asarray(W2, dtype=np.float32))
    V = np.ascontiguousarray(np.asarray(V, dtype=np.float32))
    w2_bf = W2.astype(ml_dtypes.bfloat16)
    in_maps = []
    for c in range(B):
        qc = q[c, :s_len, :]
        in_maps.append({
            "qT": np.ascontiguousarray(qc.T).astype(np_mm, copy=False),
            "qn": qc.astype(ml_dtypes.bfloat16),
            "w1": W1.astype(np_mm, copy=False),
            "vv": V.astype(np_mm, copy=False),
            "w2": w2_bf,
            "kT": np.ascontiguousarray(k[c].reshape(ND, P).T).astype(ml_dtypes.bfloat16),
        })
    return in_maps


_cached = {}


def run(q, k, W1, W2, V, trace=False):
    if "nc" not in _cached:
        _cached["nc"] = build_program(S)
    nc = _cached["nc"]
    in_maps = make_in_maps(q, k, W1, W2, V)
    res = run_bass_kernel_spmd(nc, in_maps, core_ids=list(range(B)), trace=trace)
    values = np.stack([res.results[c]["vout"][0] for c in range(B)])
    weights = np.stack([res.results[c]["wout"] for c in range(B)])
    return values, weights, res


def kernel(q, k, W1, W2, V):
    values, weights, _ = run(q, k, W1, W2, V)
    return values, weights

